# revision 1
# baseline (speedup 1.0000x reference)
"""Trainium2 Bass kernel: decoder multi-head attention (B=2, S=2048, D=1024, 16 heads).

Sharding: 8 cores = 2 batches x 4 head-groups (4 heads / 256 dims per core).
Per core (batch b, head group hg), all in transposed layouts:
  Q^T = (wq_c)^T @ xq[b]^T + bq_c      [256, 2048]
  K^T = (wk_c)^T @ xk[b]^T             [256, 2048]
  V   = xv[b] @ wv_c                   [2048, 256] token-major (no bias)
  per head h: scores^T[sk,sq] = K_h^T.T @ Q_h^T            (K=64)
              P^T = exp(scores^T/8) * mask^T               (fp16)
              [U^T; rowsum] = [V_h | 1].T @ P^T            (ones col -> rowsum)
              UT_h = U^T * (1/rowsum)  (recip via [128,8] DRAM-roundtrip reshape)
  y_partial = sum_h UT_h.T @ wf_h                          [2048, 1024]
Host: out[b] = sum_hg y_partial + bv @ wf + bf
(v bias folded out: attention rows sum to 1, so attn@(V+bv) = attn@V + bv.)
"""

import sys

if "/opt/trn_rl_repo" not in sys.path:
    sys.path.insert(0, "/opt/trn_rl_repo")

import numpy as np

B, S, D = 2, 2048, 1024
NH, DK = 16, 64
NCORES = 8
HPC = 4            # heads per core
HD = HPC * DK      # 256 head dims per core
QB = 1024          # q-block (free dim of scores^T tiles)
NQB = S // QB      # 2
NKT = S // 128     # 16 sk tiles
KC = D // 128      # 8 contraction chunks for projections

_CACHE = {}
DEBUG_DUMPS = False
BENCH_LOOP = 0     # >0: wrap body in a hardware repeat loop (for timing)
ABL_NO_MASK = False
ABL_NO_EXP = False
ABL_NO_NORM = False
GPSIMD_MASK_MOD = 0  # st % MOD == MOD-1 -> mask multiply on gpsimd (0=off)


def _build():
    import contextlib

    import concourse.mybir as mybir
    import concourse.tile as tile
    from concourse import bacc

    f32 = mybir.dt.float32
    f16 = mybir.dt.float16
    EXP = mybir.ActivationFunctionType.Exp
    IDENT = mybir.ActivationFunctionType.Identity

    nc = bacc.Bacc(
        "TRN2",
        target_bir_lowering=False,
        debug=False,
        enable_asserts=False,
        num_devices=NCORES,
    )

    xq_d = nc.dram_tensor("xqT", [128, S // 256, KC, 256], f16, kind="ExternalInput")
    xk_d = nc.dram_tensor("xkT", [128, S // 256, KC, 256], f16, kind="ExternalInput")
    xv_d = nc.dram_tensor("xvT", [128, S // 256, KC, 256], f16, kind="ExternalInput")
    wq_d = nc.dram_tensor("wq", [128, KC, HD], f16, kind="ExternalInput")
    wk_d = nc.dram_tensor("wk", [128, KC, HD], f16, kind="ExternalInput")
    wv_d = nc.dram_tensor("wv", [128, KC, HD], f16, kind="ExternalInput")
    bq_d = nc.dram_tensor("bq", [128, 2], f32, kind="ExternalInput")
    wf_d = nc.dram_tensor("wf", [64, HPC, D], f16, kind="ExternalInput")
    mk_d = nc.dram_tensor("maskT", [128, NQB, NKT, QB], f16, kind="ExternalInput")
    y_d = nc.dram_tensor("y", [S, D], f16, kind="ExternalOutput")

    with tile.TileContext(nc) as tc:
        with (
            tc.tile_pool(name="consts", bufs=1) as consts,
            tc.tile_pool(name="qk", bufs=1) as qkp,
            tc.tile_pool(name="usb", bufs=3) as usbp,
            tc.tile_pool(name="ut", bufs=6) as utp,
            tc.tile_pool(name="yo", bufs=3) as yop,
            tc.tile_pool(name="bc", bufs=3) as bcp,
            tc.tile_pool(name="rs", bufs=3) as rsp,
            tc.tile_pool(name="scr", bufs=6, space="DRAM") as scrp,
            tc.tile_pool(name="ps_s", bufs=2, space="PSUM") as ps_s,
            tc.tile_pool(name="ps_a", bufs=2, space="PSUM") as ps_a,
        ):
            # ---- constants ----
            w_sb = {}
            for name, dram in (("q", wq_d), ("k", wk_d), ("v", wv_d)):
                t = consts.tile([128, KC, HD], f16, tag=f"w{name}", name=f"w{name}")
                nc.sync.dma_start(out=t, in_=dram[:])
                w_sb[name] = t
            wf_sb = consts.tile([64, HPC, D], f16, tag="wf")
            nc.sync.dma_start(out=wf_sb, in_=wf_d[:])
            bq_sb = consts.tile([128, 2], f32, tag="bq")
            nc.sync.dma_start(out=bq_sb, in_=bq_d[:])

            # persistent activations
            QT = [qkp.tile([128, S], f16, tag=f"qt{m}", name=f"qt{m}") for m in range(2)]
            KT = [qkp.tile([128, S], f16, tag=f"kt{m}", name=f"kt{m}") for m in range(2)]
            V = [qkp.tile([128, HPC * 65], f16, tag=f"v{st}", name=f"v{st}") for st in range(NKT)]

            loop_ctx = (
                tc.For_i(0, BENCH_LOOP, 1) if BENCH_LOOP else contextlib.nullcontext()
            )
            with loop_ctx:
                NXB = 256
                with (
                    tc.tile_pool(name="xs", bufs=3) as xsp,
                    tc.tile_pool(name="mask", bufs=5) as maskp,
                    tc.tile_pool(name="pt", bufs=10) as ptp,
                    tc.tile_pool(name="exp", bufs=4) as expp,
                ):
                    # ---------- emission units ----------
                    def proj_unit(proj, m, g):
                        """One (projection, m-tile, 2-j-block group): one 1MB
                        x DMA + matmuls + evictions (DVE; idle in proj phase)."""
                        src_d = {"q": xq_d, "k": xk_d, "v": xv_d}[proj]
                        xt2 = xsp.tile([128, 2, KC, NXB], f16, tag="xs", name="xt2")
                        nc.sync.dma_start(out=xt2, in_=src_d[:][:, 2 * g : 2 * g + 2, :, :])
                        for jj in range(2):
                            j = 2 * g + jj
                            xt = xt2[:, jj, :, :]
                            if proj in ("q", "k"):
                                ps = ps_s.tile([128, QB], f32, tag="sc", name="ps")
                                for kc in range(KC):
                                    nc.tensor.matmul(
                                        ps[:, :NXB],
                                        lhsT=w_sb[proj][:, kc, m * 128 : (m + 1) * 128],
                                        rhs=xt[:, kc, :],
                                        start=(kc == 0),
                                        stop=(kc == KC - 1),
                                    )
                                dst = (QT if proj == "q" else KT)[m][
                                    :, j * NXB : (j + 1) * NXB
                                ]
                                if proj == "q":
                                    nc.scalar.activation(
                                        dst, ps[:, :NXB], IDENT, bias=bq_sb[:, m : m + 1]
                                    )
                                else:
                                    nc.scalar.copy(dst, ps[:, :NXB])
                            else:
                                for sub in range(NXB // 128):
                                    st = j * (NXB // 128) + sub
                                    ps = ps_s.tile([128, QB], f32, tag="sc", name="ps")
                                    for kc in range(KC):
                                        nc.tensor.matmul(
                                            ps[:, :HD],
                                            lhsT=xt[:, kc, sub * 128 : (sub + 1) * 128],
                                            rhs=w_sb["v"][:, kc, :],
                                            start=(kc == 0),
                                            stop=(kc == KC - 1),
                                        )
                                    vt = V[st]
                                    nc.gpsimd.memset(vt, 1.0)  # ones col at 65h+64
                                    for h in range(HPC):
                                        nc.scalar.copy(
                                            vt[:, 65 * h : 65 * h + 64],
                                            ps[:, h * 64 : (h + 1) * 64],
                                        )

                    def mask_loads(qb):
                        mts = []
                        for grp in range(NKT // 4):
                            mg = maskp.tile([128, 4, QB], f16, tag="mask", name="mg")
                            nc.scalar.dma_start(
                                out=mg, in_=mk_d[:][:, qb, 4 * grp : 4 * grp + 4, :]
                            )
                            for sub in range(4):
                                mts.append(mg[:, sub, :])
                        return mts

                    def scores(h, st, qb, mts, pts):
                        c, r = h // 2, 64 * (h % 2)
                        ps = ps_s.tile([128, QB], f32, tag="sc", name="sc")
                        for half in range(2):
                            nc.tensor.matmul(
                                ps[:, half * 512 : (half + 1) * 512],
                                lhsT=KT[c][r : r + 64, st * 128 : (st + 1) * 128],
                                rhs=QT[c][
                                    r : r + 64,
                                    qb * QB + half * 512 : qb * QB + (half + 1) * 512,
                                ],
                                start=True,
                                stop=True,
                            )
                        et = expp.tile([128, QB], f16, tag="exp", name="et")
                        if ABL_NO_EXP:
                            nc.vector.tensor_copy(et, ps)
                        else:
                            nc.scalar.activation(et, ps, EXP, scale=0.125)
                        if ABL_NO_MASK:
                            pts[(h, st)] = et
                        else:
                            pt = ptp.tile([128, QB], f16, tag="pt", name="pt")
                            eng = (
                                nc.gpsimd
                                if GPSIMD_MASK_MOD
                                and st % GPSIMD_MASK_MOD == GPSIMD_MASK_MOD - 1
                                else nc.vector
                            )
                            eng.tensor_mul(pt, et, mts[st])
                            pts[(h, st)] = pt

                    def umm(h, st, ups, pts):
                        pt = pts.pop((h, st))
                        up = ups[h]
                        for half in range(2):
                            nc.tensor.matmul(
                                up[0:65, half * 512 : (half + 1) * 512],
                                lhsT=V[st][:, 65 * h : 65 * h + 65],
                                rhs=pt[:, half * 512 : (half + 1) * 512],
                                start=(st == 0),
                                stop=(st == NKT - 1),
                            )

                    def norm(h, ups, ut_tiles):
                        # copy U+rowsum out of PSUM (releases the slot fast),
                        # recip rowsum via DRAM-roundtrip [128,8] reshape +
                        # broadcast-load, then normalize.
                        up = ups.pop(h)
                        usb = usbp.tile([128, QB], f32, tag="usb", name="usb")
                        nc.vector.tensor_copy(usb[0:65, :], up[0:65, :])
                        ut = utp.tile([64, QB], f16, tag="ut", name="ut")
                        ut_tiles[h] = ut
                        if ABL_NO_NORM:
                            nc.vector.tensor_copy(ut, usb[0:64, :])
                            return
                        scr_sum = scrp.tile([1, QB], f32, tag="scr_sum", name="scr_sum")
                        nc.gpsimd.dma_start(out=scr_sum, in_=usb[64:65, :])
                        rs2 = rsp.tile([32, QB // 32], f32, tag="rs", name="rs2")
                        nc.gpsimd.dma_start(
                            out=rs2,
                            in_=scr_sum.rearrange("a (p j) -> p (a j)", p=32),
                        )
                        nc.vector.reciprocal(out=rs2, in_=rs2)
                        scr_rcp = scrp.tile([1, QB], f32, tag="scr_rcp", name="scr_rcp")
                        nc.gpsimd.dma_start(
                            out=scr_rcp.rearrange("a (p j) -> p (a j)", p=32),
                            in_=rs2,
                        )
                        bc = bcp.tile([64, QB], f32, tag="bc", name="bc")
                        nc.gpsimd.dma_start(out=bc, in_=scr_rcp.to_broadcast([64, QB]))
                        nc.vector.tensor_mul(ut, usb[0:64, :], bc)

                    def fc_unit(qb, g, ut_tiles):
                        # two 128-row fc tiles -> one fp16 yo tile -> one DMA
                        yo = yop.tile([128, 2, D], f16, tag="yo", name="yo")
                        for jj in range(2):
                            j = 2 * g + jj
                            fp = ps_s.tile([128, QB], f32, tag="sc", name="fp")
                            for half in range(2):
                                for h in range(HPC):
                                    nc.tensor.matmul(
                                        fp[:, half * 512 : (half + 1) * 512],
                                        lhsT=ut_tiles[h][:, j * 128 : (j + 1) * 128],
                                        rhs=wf_sb[:, h, half * 512 : (half + 1) * 512],
                                        start=(h == 0),
                                        stop=(h == HPC - 1),
                                    )
                            nc.vector.tensor_copy(yo[:, jj, :], fp)
                        nc.sync.dma_start(
                            out=y_d[:][
                                qb * QB + g * 256 : qb * QB + (g + 1) * 256, :
                            ].rearrange("(r p) n -> p r n", p=128),
                            in_=yo,
                        )

                    def emit_attention(qb, mts, extra):
                        """Two pair-phases; scores of a pair are adjacent (row
                        groups 0-63/64-127 run concurrently on the PE array);
                        U matmuls lag scores by 2 tiles so the exp/mask chain
                        stays off the PE critical path. Extra units (deferred
                        m1 projections for qb0, previous q-block's fc for
                        qb1+) ride inside the pair loops."""
                        pts, ut_tiles, ups = {}, {}, {}
                        for pair in range(2):
                            h0, h1 = 2 * pair, 2 * pair + 1
                            ups[h0] = ps_a.tile([128, QB], f32, tag="acc", name="upA")
                            ups[h1] = ps_a.tile([128, QB], f32, tag="acc", name="upB")
                            for st in range(NKT + 2):
                                if st < NKT:
                                    scores(h0, st, qb, mts, pts)
                                    scores(h1, st, qb, mts, pts)
                                if st >= 2:
                                    umm(h0, st - 2, ups, pts)
                                    umm(h1, st - 2, ups, pts)
                                if extra and (pair + st) % 2 == 1:
                                    extra.pop(0)()
                            norm(h0, ups, ut_tiles)
                            norm(h1, ups, ut_tiles)
                        for t in extra:
                            t()
                        return ut_tiles

                    # ---------- main emission ----------
                    for g in range(S // NXB // 2):
                        proj_unit("q", 0, g)
                    for g in range(S // NXB // 2):
                        proj_unit("q", 1, g)
                    for g in range(S // NXB // 2):
                        proj_unit("k", 0, g)
                    for g in range(S // NXB // 2):
                        proj_unit("k", 1, g)
                    for g in range(S // NXB // 2):
                        proj_unit("v", None, g)

                    uts = None
                    for qb in range(NQB):
                        mts = mask_loads(qb)
                        extra = (
                            [
                                lambda g=g, u=uts: fc_unit(qb - 1, g, u)
                                for g in range(QB // 256)
                            ]
                            if uts is not None
                            else []
                        )
                        uts = emit_attention(qb, mts, extra)
                    for g in range(QB // 256):
                        fc_unit(NQB - 1, g, uts)

    nc.compile()
    return nc


def get_nc():
    if "nc" not in _CACHE:
        _CACHE["nc"] = _build()
    return _CACHE["nc"]


def make_in_maps(q, k, v, mask, wq, bq, wk, wv, wf):
    q = np.asarray(q, np.float32)
    k = np.asarray(k, np.float32)
    v = np.asarray(v, np.float32)
    def tile_x(x):
        # [S, D] -> x^T tiled as [128, S/256, KC, 256]:
        # element (c*128+p, j*256+s) -> [p, j, c, s]
        xt = x.T.astype(np.float16).reshape(KC, 128, S // 256, 256)
        return np.ascontiguousarray(xt.transpose(1, 2, 0, 3))

    xqT = [tile_x(q[b]) for b in range(B)]
    xkT = [tile_x(k[b]) for b in range(B)]
    xvT = [tile_x(v[b]) for b in range(B)]
    def tile_mask(m):
        # mask^T [sk, sq] -> [128, NQB, NKT, QB]: (st*128+p, qb*QB+s) -> [p, qb, st, s]
        mt = m.T.astype(np.float16).reshape(NKT, 128, NQB, QB)
        return np.ascontiguousarray(mt.transpose(1, 2, 0, 3))

    mkT = [tile_mask(np.asarray(mask[b])) for b in range(B)]
    wq = np.asarray(wq, np.float16)
    wk = np.asarray(wk, np.float16)
    wv = np.asarray(wv, np.float16)
    wf = np.asarray(wf, np.float16)
    bq = np.asarray(bq, np.float32)
    in_maps = []
    for c in range(NCORES):
        b, hg = c // HPC, c % HPC
        cols = slice(hg * HD, (hg + 1) * HD)
        in_maps.append(
            {
                "xqT": xqT[b],
                "xkT": xkT[b],
                "xvT": xvT[b],
                "wq": np.ascontiguousarray(
                    wq[:, cols].reshape(KC, 128, HD).transpose(1, 0, 2)
                ),
                "wk": np.ascontiguousarray(
                    wk[:, cols].reshape(KC, 128, HD).transpose(1, 0, 2)
                ),
                "wv": np.ascontiguousarray(
                    wv[:, cols].reshape(KC, 128, HD).transpose(1, 0, 2)
                ),
                "bq": np.ascontiguousarray(bq[cols].reshape(2, 128).T),
                "wf": np.ascontiguousarray(
                    wf[cols, :].reshape(HPC, 64, D).transpose(1, 0, 2)
                ),
                "maskT": mkT[b],
            }
        )
    return in_maps


LAST_RESULTS = None


def kernel(q, k, v, mask, wq, bq, wk, wv, bv, wf, bf, **trace_kwargs):
    from concourse.bass_utils import run_bass_kernel_spmd

    global LAST_RESULTS
    nc = get_nc()
    in_maps = make_in_maps(q, k, v, mask, wq, bq, wk, wv, wf)
    res = run_bass_kernel_spmd(
        nc, in_maps, core_ids=list(range(NCORES)), **trace_kwargs
    )
    LAST_RESULTS = res
    out = np.zeros((B, S, D), np.float64)
    for c in range(NCORES):
        out[c // HPC] += res.results[c]["y"].astype(np.float64)
    extra = (
        np.asarray(bv, np.float64) @ np.asarray(wf, np.float64)
        + np.asarray(bf, np.float64)
    )
    out += extra[None, None, :]
    return out.astype(np.float32)



# revision 13
# speedup vs baseline: 1.2078x; 1.2078x over previous
"""Trainium2 Bass kernel: decoder multi-head attention (B=2, S=2048, D=1024, 16 heads).

Sharding: 8 cores = 2 batches x 4 head-groups (4 heads / 256 dims per core).
Per core (batch b, head group hg), all in transposed layouts:
  Q^T = (wq_c)^T @ xq[b]^T + bq_c      [256, 2048]
  K^T = (wk_c)^T @ xk[b]^T             [256, 2048]
  V   = xv[b] @ wv_c                   [2048, 256] token-major (no bias)
  per head h: scores^T[sk,sq] = K_h^T.T @ Q_h^T            (K=64, head pairs
              run concurrently on PE row groups 0-63/64-127)
              P^T = exp(scores^T/8) * mask^T               (fp16)
              [U^T; rowsum] = [V_h | 1].T @ P^T            (ones col -> rowsum)
  per pair p: UT2_p[128,sq] = [U_h0; U_h1] * (1/rowsum)    (recip_approx_fast
              on PSUM row 64 + gpsimd partition_broadcast; no DRAM roundtrip)
  y_partial = sum_p UT2_p.T @ wf2_p                        [2048, 1024]
              (head-pair stacked: contraction 128)
Host: out[b] = sum_hg y_partial + bv @ wf + bf
(v bias folded out: attention rows sum to 1, so attn@(V+bv) = attn@V + bv.)

Scheduling: x tiles DMA'd once per token group (shared by both w row tiles);
all mask DMAs issued up front on the gpsimd queue; PSUM evictions on DVE
(ACT does exp only); q-proj for the second q-block and fc for the previous
q-block ride in the attention pair-boundary bubbles.
"""

import sys

if "/opt/trn_rl_repo" not in sys.path:
    sys.path.insert(0, "/opt/trn_rl_repo")

import numpy as np

B, S, D = 2, 2048, 1024
NH, DK = 16, 64
NCORES = 8
HPC = 4            # heads per core
HD = HPC * DK      # 256 head dims per core
QB = 1024          # q-block (free dim of scores^T tiles)
NQB = S // QB      # 2
NKT = S // 128     # 16 sk tiles
KC = D // 128      # 8 contraction chunks for projections

_CACHE = {}


def _build():
    import concourse.mybir as mybir
    import concourse.tile as tile
    from concourse import bacc

    f32 = mybir.dt.float32
    f16 = mybir.dt.float16
    EXP = mybir.ActivationFunctionType.Exp

    nc = bacc.Bacc(
        "TRN2",
        target_bir_lowering=False,
        debug=False,
        enable_asserts=False,
        num_devices=NCORES,
    )

    xq_d = nc.dram_tensor("xqT", [128, S // 256, KC, 256], f16, kind="ExternalInput")
    xk_d = nc.dram_tensor("xkT", [128, S // 256, KC, 256], f16, kind="ExternalInput")
    xv_d = nc.dram_tensor("xvT", [128, S // 256, KC, 256], f16, kind="ExternalInput")
    wq_d = nc.dram_tensor("wq", [128, KC, HD], f16, kind="ExternalInput")
    wk_d = nc.dram_tensor("wk", [128, KC, HD], f16, kind="ExternalInput")
    wv_d = nc.dram_tensor("wv", [128, KC, HD], f16, kind="ExternalInput")
    bq_d = nc.dram_tensor("bq", [128, 2], f32, kind="ExternalInput")
    wf_d = nc.dram_tensor("wf", [128, 2, D], f16, kind="ExternalInput")
    mk_d = nc.dram_tensor("maskT", [128, NQB, NKT, QB], f16, kind="ExternalInput")
    y_d = nc.dram_tensor("y", [S, D], f16, kind="ExternalOutput")

    with tile.TileContext(nc) as tc:
        with (
            tc.tile_pool(name="consts", bufs=1) as consts,
            tc.tile_pool(name="qk", bufs=1) as qkp,
            tc.tile_pool(name="maskc", bufs=1) as maskc,
            tc.tile_pool(name="xs", bufs=3) as xsp,
            tc.tile_pool(name="exp", bufs=5) as expp,
            tc.tile_pool(name="pt", bufs=10) as ptp,
            tc.tile_pool(name="usb", bufs=2) as usbp,
            tc.tile_pool(name="rs", bufs=2) as rsp,
            tc.tile_pool(name="rs2", bufs=2) as rs2p,
            tc.tile_pool(name="bc", bufs=2) as bcp,
            tc.tile_pool(name="scr", bufs=4, space="DRAM") as scrp,
            tc.tile_pool(name="ut", bufs=4) as utp,
            tc.tile_pool(name="yo", bufs=2) as yop,
            tc.tile_pool(name="ps_s", bufs=2, space="PSUM") as ps_s,
            tc.tile_pool(name="ps_a", bufs=2, space="PSUM") as ps_a,
        ):
            # ---- constants (k first: k-proj units run first) ----
            w_sb = {}
            for name, dram in (("k", wk_d), ("v", wv_d), ("q", wq_d)):
                t = consts.tile([128, KC, HD], f16, tag=f"w{name}", name=f"w{name}")
                nc.sync.dma_start(out=t, in_=dram[:])
                w_sb[name] = t
            bq_sb = consts.tile([128, 2], f32, tag="bq")
            nc.sync.dma_start(out=bq_sb, in_=bq_d[:])
            wf_sb = consts.tile([128, 2, D], f16, tag="wf")
            nc.sync.dma_start(out=wf_sb, in_=wf_d[:])

            # ---- all mask tiles prefetched on the gpsimd queue ----
            mtiles = {}
            for qb in range(NQB):
                for grp in range(NKT // 4):
                    mg = maskc.tile(
                        [128, 4, QB], f16, tag=f"m{qb}_{grp}", name=f"m{qb}_{grp}"
                    )
                    nc.gpsimd.dma_start(
                        out=mg, in_=mk_d[:][:, qb, 4 * grp : 4 * grp + 4, :]
                    )
                    mtiles[(qb, grp)] = mg

            # persistent activations
            QT = [qkp.tile([128, S], f16, tag=f"qt{m}", name=f"qt{m}") for m in range(2)]
            KT = [qkp.tile([128, S], f16, tag=f"kt{m}", name=f"kt{m}") for m in range(2)]
            V = [qkp.tile([128, HPC * 65], f16, tag=f"v{st}", name=f"v{st}") for st in range(NKT)]

            # ---------- emission units ----------
            def proj_qk_unit(proj, g):
                """One token group (512 tokens) of q/k projection: one 1MB x
                DMA shared by both 128-row weight tiles; evictions on DVE."""
                src_d = xq_d if proj == "q" else xk_d
                xt2 = xsp.tile([128, 2, KC, 256], f16, tag="xs", name="xt2")
                nc.sync.dma_start(out=xt2, in_=src_d[:][:, 2 * g : 2 * g + 2, :, :])
                for m in range(2):
                    for jj in range(2):
                        j = 2 * g + jj
                        xt = xt2[:, jj, :, :]
                        ps = ps_s.tile([128, QB], f32, tag="sc", name="ps")
                        for kc in range(KC):
                            nc.tensor.matmul(
                                ps[:, :256],
                                lhsT=w_sb[proj][:, kc, m * 128 : (m + 1) * 128],
                                rhs=xt[:, kc, :],
                                start=(kc == 0),
                                stop=(kc == KC - 1),
                            )
                        dst = (QT if proj == "q" else KT)[m][:, j * 256 : (j + 1) * 256]
                        if proj == "q":
                            nc.vector.tensor_scalar_add(dst, ps[:, :256], bq_sb[:, m : m + 1])
                        else:
                            nc.vector.tensor_copy(dst, ps[:, :256])

            def proj_v_unit(g):
                """One token group of v projection (4 sk tiles, token-major)."""
                xt2 = xsp.tile([128, 2, KC, 256], f16, tag="xs", name="xt2")
                nc.sync.dma_start(out=xt2, in_=xv_d[:][:, 2 * g : 2 * g + 2, :, :])
                for jj in range(2):
                    for sub in range(2):
                        st = (2 * g + jj) * 2 + sub
                        ps = ps_s.tile([128, QB], f32, tag="sc", name="ps")
                        for kc in range(KC):
                            nc.tensor.matmul(
                                ps[:, :HD],
                                lhsT=xt2[:, jj, kc, sub * 128 : (sub + 1) * 128],
                                rhs=w_sb["v"][:, kc, :],
                                start=(kc == 0),
                                stop=(kc == KC - 1),
                            )
                        vt = V[st]
                        vt3 = vt.rearrange("p (h c) -> p h c", h=HPC)
                        nc.gpsimd.memset(vt3[:, :, 64:65], 1.0)
                        nc.vector.tensor_copy(
                            vt3[:, :, 0:64],
                            ps[:, :HD].rearrange("p (h c) -> p h c", h=HPC),
                        )

            def scores(h, st, qb, pts):
                c, r = h // 2, 64 * (h % 2)
                ps = ps_s.tile([128, QB], f32, tag="sc", name="sc")
                for half in range(2):
                    nc.tensor.matmul(
                        ps[:, half * 512 : (half + 1) * 512],
                        lhsT=KT[c][r : r + 64, st * 128 : (st + 1) * 128],
                        rhs=QT[c][
                            r : r + 64,
                            qb * QB + half * 512 : qb * QB + (half + 1) * 512,
                        ],
                        start=True,
                        stop=True,
                    )
                et = expp.tile([128, QB], f16, tag="exp", name="et")
                nc.scalar.activation(et, ps, EXP, scale=0.125)
                pt = ptp.tile([128, QB], f16, tag="pt", name="pt")
                nc.vector.tensor_mul(pt, et, mtiles[(qb, st // 4)][:, st % 4, :])
                pts[(h, st)] = pt

            def umm(h, st, ups, pts):
                pt = pts.pop((h, st))
                up = ups[h]
                for half in range(2):
                    nc.tensor.matmul(
                        up[0:65, half * 512 : (half + 1) * 512],
                        lhsT=V[st][:, 65 * h : 65 * h + 65],
                        rhs=pt[:, half * 512 : (half + 1) * 512],
                        start=(st == 0),
                        stop=(st == NKT - 1),
                    )

            def norm_pair_evict(pair, ups, state):
                """Pair-end PSUM readout (frees the accumulators fast): U of
                both heads stacked into one [128, QB] tile, rowsums into rows
                0/32 of a small staging tile. The off-chip reciprocal chain is
                deferred (norm_pair_finish) so it never blocks the DVE queue."""
                h0, h1 = 2 * pair, 2 * pair + 1
                up0, up1 = ups.pop(h0), ups.pop(h1)
                usb = usbp.tile([128, QB], f32, tag="usb", name="usb")
                nc.vector.tensor_copy(usb[0:64, :], up0[0:64, :])
                nc.vector.tensor_copy(usb[64:128, :], up1[0:64, :])
                rs = rsp.tile([33, QB], f32, tag="rs", name="rs")
                nc.vector.tensor_copy(rs[0:1, :], up0[64:65, :])
                nc.vector.tensor_copy(rs[32:33, :], up1[64:65, :])
                state["usb"], state["rs"] = usb, rs

            def norm_pair_finish(gp, state, ut_pairs):
                """Deferred: rowsums -> DRAM -> [64,32] reshape -> reciprocal
                -> DRAM -> broadcast-DMA into both 64-row halves -> one
                full-tile gpsimd multiply."""
                usb, rs = state.pop("usb"), state.pop("rs")
                scr = scrp.tile([2, QB], f32, tag="scr", name="scr")
                nc.gpsimd.dma_start(out=scr[0:1, :], in_=rs[0:1, :])
                nc.gpsimd.dma_start(out=scr[1:2, :], in_=rs[32:33, :])
                rs2 = rs2p.tile([64, 2, QB // 64], f32, tag="rs2", name="rs2")
                nc.gpsimd.dma_start(
                    out=rs2, in_=scr.rearrange("a (p j) -> p a j", p=64)
                )
                nc.vector.reciprocal(out=rs2, in_=rs2)
                scr2 = scrp.tile([2, QB], f32, tag="scr2", name="scr2")
                nc.gpsimd.dma_start(
                    out=scr2.rearrange("a (p j) -> p a j", p=64), in_=rs2
                )
                bc = bcp.tile([128, QB], f32, tag="bc", name="bc")
                nc.gpsimd.dma_start(out=bc[0:64, :], in_=scr2[0:1, :].to_broadcast([64, QB]))
                nc.gpsimd.dma_start(out=bc[64:128, :], in_=scr2[1:2, :].to_broadcast([64, QB]))
                ut2 = utp.tile([128, QB], f16, tag="ut", name="ut2")
                nc.gpsimd.tensor_mul(ut2, usb, bc)
                ut_pairs[gp] = ut2

            def fc_unit(qb, g, ut_get):
                # two 128-row fc tiles -> one fp16 yo tile -> one DMA
                yo = yop.tile([128, 2, D], f16, tag="yo", name="yo")
                for jj in range(2):
                    j = 2 * g + jj
                    fp = ps_s.tile([128, QB], f32, tag="sc", name="fp")
                    for half in range(2):
                        for p in range(2):
                            nc.tensor.matmul(
                                fp[:, half * 512 : (half + 1) * 512],
                                lhsT=ut_get(p)[:, j * 128 : (j + 1) * 128],
                                rhs=wf_sb[:, p, half * 512 : (half + 1) * 512],
                                start=(p == 0),
                                stop=(p == 1),
                            )
                    nc.vector.tensor_copy(yo[:, jj, :], fp)
                nc.sync.dma_start(
                    out=y_d[:][
                        qb * QB + g * 256 : qb * QB + (g + 1) * 256, :
                    ].rearrange("(r p) n -> p r n", p=128),
                    in_=yo,
                )

            def emit_attention(qb, extras, ut_pairs, deferred):
                """Two pair-phases; scores of a pair are adjacent (row groups
                0-63/64-127 run concurrently on the PE array); U matmuls lag
                scores by 2 tiles. Extra units (q-proj for qb1, previous
                q-block's fc) fill the pair-boundary bubbles where the PE
                waits on the normalization readout of the previous pair.
                The off-chip part of each pair's normalization is emitted one
                pair later (popped from `deferred`)."""
                ups, pts = {}, {}
                for pair in range(2):
                    h0, h1 = 2 * pair, 2 * pair + 1
                    ups[h0] = ps_a.tile([128, QB], f32, tag="acc", name="upA")
                    ups[h1] = ps_a.tile([128, QB], f32, tag="acc", name="upB")
                    for st in range(NKT + 2):
                        if st == 0 and deferred:
                            deferred.pop(0)()
                        if st < 2 and extras:
                            extras.pop(0)()
                        if st < NKT:
                            scores(h0, st, qb, pts)
                            scores(h1, st, qb, pts)
                        if st >= 2:
                            umm(h0, st - 2, ups, pts)
                            umm(h1, st - 2, ups, pts)
                    state = {}
                    norm_pair_evict(pair, ups, state)
                    gp = (qb, pair)
                    deferred.append(
                        lambda gp=gp, state=state: norm_pair_finish(gp, state, ut_pairs)
                    )
                for t in extras:
                    t()

            # ---------- main emission ----------
            for g in range(4):
                proj_qk_unit("k", g)
            for g in range(4):
                proj_v_unit(g)
            proj_qk_unit("q", 0)
            proj_qk_unit("q", 1)

            ut_pairs, deferred = {}, []
            emit_attention(
                0,
                [lambda: proj_qk_unit("q", 2), lambda: proj_qk_unit("q", 3)],
                ut_pairs,
                deferred,
            )
            uts0 = lambda p: ut_pairs[(0, p)]
            emit_attention(
                1,
                [lambda g=g: fc_unit(0, g, uts0) for g in range(4)],
                ut_pairs,
                deferred,
            )
            for t in deferred:
                t()
            uts1 = lambda p: ut_pairs[(1, p)]
            for g in range(4):
                fc_unit(1, g, uts1)

    nc.compile()
    return nc


def get_nc():
    if "nc" not in _CACHE:
        _CACHE["nc"] = _build()
    return _CACHE["nc"]


def make_in_maps(q, k, v, mask, wq, bq, wk, wv, wf):
    q = np.asarray(q, np.float32)
    k = np.asarray(k, np.float32)
    v = np.asarray(v, np.float32)
    def tile_x(x):
        # [S, D] -> x^T tiled as [128, S/256, KC, 256]:
        # element (c*128+p, j*256+s) -> [p, j, c, s]
        xt = x.T.astype(np.float16).reshape(KC, 128, S // 256, 256)
        return np.ascontiguousarray(xt.transpose(1, 2, 0, 3))

    xqT = [tile_x(q[b]) for b in range(B)]
    xkT = [tile_x(k[b]) for b in range(B)]
    xvT = [tile_x(v[b]) for b in range(B)]
    def tile_mask(m):
        # mask^T [sk, sq] -> [128, NQB, NKT, QB]: (st*128+p, qb*QB+s) -> [p, qb, st, s]
        mt = m.T.astype(np.float16).reshape(NKT, 128, NQB, QB)
        return np.ascontiguousarray(mt.transpose(1, 2, 0, 3))

    mkT = [tile_mask(np.asarray(mask[b])) for b in range(B)]
    wq = np.asarray(wq, np.float16)
    wk = np.asarray(wk, np.float16)
    wv = np.asarray(wv, np.float16)
    wf = np.asarray(wf, np.float16)
    bq = np.asarray(bq, np.float32)
    in_maps = []
    for c in range(NCORES):
        b, hg = c // HPC, c % HPC
        cols = slice(hg * HD, (hg + 1) * HD)
        in_maps.append(
            {
                "xqT": xqT[b],
                "xkT": xkT[b],
                "xvT": xvT[b],
                "wq": np.ascontiguousarray(
                    wq[:, cols].reshape(KC, 128, HD).transpose(1, 0, 2)
                ),
                "wk": np.ascontiguousarray(
                    wk[:, cols].reshape(KC, 128, HD).transpose(1, 0, 2)
                ),
                "wv": np.ascontiguousarray(
                    wv[:, cols].reshape(KC, 128, HD).transpose(1, 0, 2)
                ),
                "bq": np.ascontiguousarray(bq[cols].reshape(2, 128).T),
                # head-pair stacked fc weights: [128 rows of pair p, p, D]
                "wf": np.ascontiguousarray(
                    wf[cols, :].reshape(2, 128, D).transpose(1, 0, 2)
                ),
                "maskT": mkT[b],
            }
        )
    return in_maps


LAST_RESULTS = None


def kernel(q, k, v, mask, wq, bq, wk, wv, bv, wf, bf, **trace_kwargs):
    from concourse.bass_utils import run_bass_kernel_spmd

    global LAST_RESULTS
    nc = get_nc()
    in_maps = make_in_maps(q, k, v, mask, wq, bq, wk, wv, wf)
    res = run_bass_kernel_spmd(
        nc, in_maps, core_ids=list(range(NCORES)), **trace_kwargs
    )
    LAST_RESULTS = res
    out = np.zeros((B, S, D), np.float64)
    for c in range(NCORES):
        out[c // HPC] += res.results[c]["y"].astype(np.float64)
    extra = (
        np.asarray(bv, np.float64) @ np.asarray(wf, np.float64)
        + np.asarray(bf, np.float64)
    )
    out += extra[None, None, :]
    return out.astype(np.float32)
